# revision 81
# baseline (speedup 1.0000x reference)
"""CARAFE content-aware upsampling on 8 Trainium2 NeuronCores (Bass/Tile).

Problem: x[2,256,64,64], 1x1 compress conv (256->32), 5x5 encoder conv
(32->100), pixel-shuffle(r=2) + softmax over 25 taps, then dynamic-filter
reassembly: out[b,c,2h+r1,2w+r2] = sum_k x[b,c,h+di,w+dj] * softmax_w.

Sharding: pure data-parallel over (batch, 16-row H slices) -> 8 cores.

Per-core mapping (all DMA rides one SP HWDGE queue in need-order; the
cost model serializes descriptor-gen ~630ns each on the shared HWDGE
device and charges 900ns completion-semaphore latency per DMA, so loads
are packed/coalesced and chunked only where arrival time gates compute):
  - Host ships x twice, both bf16: channel-major (packed with the tiny
    weights into a "head" + "tail" DMA pair so compress starts ~3.3us)
    and window-major xcall [120, 8192] holding the overlapping 6x20 MAC
    stationaries, so no on-device transpose/gather.
  - compress conv writes a 4-high column-shifted y1 stack (bf16), so the
    5x5 encoder conv contracts 4 dj taps per K=128 matmul (plus a K=32
    dj=4 pass): 10 matmuls per unit.
  - The back half is pipelined as 4 units U=(ro, hh): encoder conv ->
    softmax (channel-major, select-matrix matmuls for tap sums and
    reciprocal broadcast) -> band build -> MAC -> store. A unit's MAC
    only needs its own h-half band columns, so outputs stream from ~11us.
  - Band build reads the normalized weights yM [o''=sub*32+tap, pix]
    directly with partition-offset moving APs (4 sub-slices per wi), so
    no tap-major relayout copies; the block-sparse band ybig [120, 4096]
    (bf16) is written by tiny PE matmuls against host-prepared 0/1
    placement matrices (matmul writes the band's zeros too).
  - The 25-tap dynamic-filter sum runs on PE as bf16 [120]x[128] matmuls
    (stationary = xcall windows, moving = strided ybig views); psum
    halves are copied bf16 to osb on alternating Act/DVE and stored per
    (ro, ct, hh) as 8 [128,1024] DMAs; host upcasts/unscrambles.
  - A chain of dummy matmuls at t=0 ramps the PE p-state before real
    work arrives.
"""

import os
import sys

# Lift the byte-range dependency-analysis work cap so strided/partition-
# crossing APs get precise (not whole-tensor) dependency edges at build time.
os.environ.setdefault("TILE_EXHAUSTIVE_MEMORY_SHARE_CHECK", "1")

sys.path.insert(0, "/opt/trn_rl_repo")

import numpy as np
import ml_dtypes

import concourse.bacc as bacc
import concourse.bass as bass
import concourse.tile as tile
from concourse import mybir
from concourse.ap import AP

F32 = mybir.dt.float32
F32R = mybir.dt.float32r
BF16 = mybir.dt.bfloat16
BF16NP = ml_dtypes.bfloat16

# geometry
B, C, H, W = 2, 256, 64, 64
RATIO, K_UP, C_MID, ENC_K = 2, 5, 32, 5
NK = RATIO * RATIO * K_UP * K_UP  # 100
HSLICE = 16                       # output source rows per core
ROWS = HSLICE + 4                 # with 2-row halo each side
WP = W + 4                        # padded width
PADPOS = ROWS * WP                # 1360
NCORES = 8
KDIM = 120                        # 6x20 window pixels per row-pair block
YF = 4096                         # band matrix columns
NPRIME = 24                       # PE p-state priming matmuls

XB0 = 816        # head DMA covers x cols [0, 816) = rows 0-11
TOFF = 780       # tail DMA covers x cols [780, 1361] (overlap + pad col)
XT = 581         # tail cols per channel-tile
HEADC = 68 + 2 * XB0       # wp128 | selb | x0 head | x1 head
TAILC = 2 * XT
# compress chunks: chunk 0 fits the first head DMA (x cols <= 415),
# chunk 1 the second (<= 815); psum-bank limit keeps each <= 512 wide
CHUNKS = [(0, 414), (414, 400), (814, 273), (1087, PADPOS - 1087)]


def build_program(with_ebias: bool):
    nc = bacc.Bacc()
    head_d = nc.declare_dram_parameter("headp", [128, HEADC], BF16, isOutput=False)
    tail_d = nc.declare_dram_parameter("tailp", [128, TAILC], BF16, isOutput=False)
    wet_d = nc.declare_dram_parameter("wenc", [128, 1280], BF16, isOutput=False)
    pp_d = nc.declare_dram_parameter("ppackt", [25, 3968], BF16, isOutput=False)
    xc_d = nc.declare_dram_parameter("xcall", [KDIM, 8192], BF16, isOutput=False)
    if with_ebias:
        ebias_d = nc.declare_dram_parameter("ebias", [2, 128, 512], F32, isOutput=False)
    out_d = nc.declare_dram_parameter("out", [2, 128, YF], BF16, isOutput=True)

    with tile.TileContext(nc) as tc:
        # Partition-crossing APs confuse the byte-range race detector; deps
        # are tracked at tensor granularity regardless.
        tc.race_detector_enabled = False
        with (
            tc.tile_pool(name="persist", bufs=1) as pp,
            tc.tile_pool(name="psY", bufs=2, space="PSUM") as psY,   # prime/enc/sum/bcast
        ):
            psM = None  # opened after the compress pool closes (bank budget)
            # ---- PE p-state priming ----
            dummy = pp.tile([128, 128], BF16, tag="dummy")
            nc.gpsimd.memset(dummy[:], 0.0)
            for _ in range(NPRIME):
                ps = psY.tile([128, 256], F32, tag="y")
                nc.tensor.matmul(
                    ps[:, 0:128], dummy[:], dummy[:], start=True, stop=True
                )

            # ---- input loads (SP HWDGE queue, need-order) ----
            head = pp.tile([128, HEADC], BF16, tag="head")
            nc.sync.dma_start(head[:, 0:900], head_d[:, 0:900])
            nc.sync.dma_start(head[:, 900:], head_d[:, 900:])
            tail = pp.tile([128, TAILC], BF16, tag="tail")
            nc.sync.dma_start(tail[:], tail_d[:])
            wenc = pp.tile([128, 1280], BF16, tag="wenc")
            nc.sync.dma_start(wenc[:], wet_d[:])
            ppk = pp.tile([25, 3968], BF16, tag="ppackt")
            nc.sync.dma_start(ppk[:], pp_d[:])
            if with_ebias:
                ebias = []
                for ro in range(2):
                    t = pp.tile([128, 512], F32, name=f"ebias{ro}", tag=f"ebias{ro}")
                    nc.sync.dma_start(t[:], ebias_d[ro])
                    ebias.append(t)
            xcall = pp.tile([KDIM, 8192], BF16, tag="xcall")
            nc.sync.dma_start(xcall[:, 0:4096], xc_d[:, 0:4096])
            nc.sync.dma_start(xcall[:, 4096:], xc_d[:, 4096:])

            def xsrc(ct, col, n):
                """stride-2 AP into the x0/x1 column-interleaved head/tail."""
                if col + n <= XB0:
                    return AP(head.tensor, 68 + 2 * col + ct,
                              [[HEADC, 128], [2, n]])
                assert col >= TOFF
                return AP(tail.tensor, 2 * (col - TOFF) + ct,
                          [[TAILC, 128], [2, n]])

            # ---- compress conv -> 4-high column-shifted stack ----
            stk = pp.tile([128, PADPOS], BF16, tag="stk")
            ctx_cmp = tc.tile_pool(name="psC", bufs=3, space="PSUM")
            psC = ctx_cmp.__enter__()
            for ci, (off, n) in enumerate(CHUNKS):
                # both column-shifts land in one psum tile (partition blocks
                # 0/32), so a single [64, n] copy fills stack blocks 0-1
                ps = psC.tile([128, 512], F32, tag="c")
                for b in range(2):
                    for ct in range(2):
                        nc.tensor.matmul(
                            ps[32 * b:32 * b + C_MID, :n],
                            head[:, 32 * ct:32 * ct + 32],
                            xsrc(ct, off + b, n),
                            start=(ct == 0), stop=(ct == 1),
                        )
                if ci == 1:
                    # chunk 1's copy gates the first encoder unit: split it
                    # across both engines to halve the latency
                    h1 = n // 2
                    nc.vector.tensor_copy(stk[0:64, off:off + h1], ps[0:64, :h1])
                    nc.scalar.copy(stk[0:64, off + h1:off + n], ps[0:64, h1:n])
                else:
                    eng = (nc.vector.tensor_copy, nc.scalar.copy)[ci % 2]
                    eng(stk[0:64, off:off + n], ps[0:64, :n])
                # blocks 2/3: +2-shifted copies of blocks 0/1 (DVE 4x mode)
                s0 = max(0, off - 2)
                s1 = off + n - 2
                nc.vector.tensor_copy(stk[64:128, s0:s1], stk[0:64, s0 + 2:s1 + 2])
                if ci == len(CHUNKS) - 1:
                    nc.vector.tensor_copy(
                        stk[64:128, s1:s1 + 2], stk[0:64, s1:s1 + 2]
                    )
            ctx_cmp.__exit__(None, None, None)
            ctx_mac = tc.tile_pool(name="psM", bufs=3, space="PSUM")
            psM = ctx_mac.__enter__()

            # ---- pipelined back half: 4 units U = (ro, hh) ----
            # unit state
            y2e = {}
            rs = {}
            yM = {}
            ebias_sl = {}
            ybig = pp.tile([KDIM, YF], BF16, tag="ybig")
            osbs = {}

            pses = {}

            def unit_enc_part(ro, hh, part_sel):
                """encoder conv psum matmuls (half of them per call)."""
                u = f"{ro}{hh}"
                if part_sel == 0:
                    pses[(ro, hh)] = psY.tile(
                        [128, 256], F32, name=f"pse{ro}{hh}", tag="y"
                    )
                pse = pses[(ro, hh)]
                for i in range(5):
                    part = 1 - part_sel
                    di = i
                    nmm = part_sel * 5 + i
                    if part == 0:
                        lhsT = wenc[:, di * 128:di * 128 + 128]
                        kp = 128
                    else:
                        lhsT = wenc[0:32, 640 + di * 128:640 + di * 128 + 128]
                        kp = 32
                    rhs = AP(
                        stk.tensor,
                        (ro + di) * WP + 4 * part + hh * 8 * WP,
                        [[PADPOS, kp], [1, 16], [2 * WP, 4], [16, 4]],
                    )
                    nc.tensor.matmul(
                        pse[0:128, 0:256], lhsT, rhs,
                        start=(nmm == 0), stop=(nmm == 9),
                    )

            def unit_exp(ro, hh):
                """exp of the completed encoder psum (Act)."""
                u = f"{ro}{hh}"
                pse = pses[(ro, hh)]
                t = pp.tile([128, 256], BF16, name="y2e" + u, tag="y2e" + u)
                if with_ebias:
                    y2f = pp.tile([128, 256], F32, name="y2f" + u, tag="y2f" + u)
                    nc.vector.scalar_tensor_tensor(
                        y2f[:], pse[0:128, 0:256], 1.0,
                        AP(ebias[ro].tensor, hh * 16, [[512, 128], [32, 16], [1, 16]]),
                        op0=mybir.AluOpType.mult, op1=mybir.AluOpType.add,
                    )
                    nc.scalar.activation(
                        t[:], y2f[:], mybir.ActivationFunctionType.Exp
                    )
                else:
                    nc.scalar.activation(
                        t[:], pse[0:128, 0:256], mybir.ActivationFunctionType.Exp
                    )
                y2e[(ro, hh)] = t

            def unit_sum(ro, hh):
                """tap-sum select matmul + reciprocal (DVE)."""
                u = f"{ro}{hh}"
                pss = psY.tile([128, 256], F32, tag="y")
                nc.tensor.matmul(
                    pss[0:4, 0:256], head[:, 64:68], y2e[(ro, hh)][:],
                    start=True, stop=True,
                )
                r = pp.tile([4, 256], BF16, name="rs" + u, tag="rs" + u)
                with nc.allow_low_precision(reason="softmax sum recip in bf16"):
                    nc.vector.reciprocal(r[:], pss[0:4, 0:256])
                rs[(ro, hh)] = r

            def unit_bcast(ro, hh):
                """recip broadcast matmul + normalize TT (DVE)."""
                u = f"{ro}{hh}"
                psb = psY.tile([128, 256], F32, tag="y")
                nc.tensor.matmul(
                    psb[0:128, 0:256], ppk[0:4, 3840:3968], rs[(ro, hh)][:],
                    start=True, stop=True,
                )
                t = pp.tile([128, 256], BF16, name="yM" + u, tag="yM" + u)
                nc.vector.tensor_tensor(
                    t[:], y2e[(ro, hh)][:], psb[0:128, 0:256],
                    op=mybir.AluOpType.mult,
                )
                # tap-major relayout (matmul operands must share a base
                # partition, so the sub-slices move to partitions 0-24):
                # ymp[tap, wi*64 + sub*16 + j] = yM[32sub+tap, wi*16+j]
                mp = pp.tile([25, 1024], BF16, name="ymp" + u, tag="ymp" + u)
                for sub in range(4):
                    # 2 on DVE (4x) + 2 on the otherwise-idle Pool: Act/DVE
                    # copy throughput paces the tail, Pool does not
                    eng = (nc.vector.tensor_copy, nc.gpsimd.tensor_copy)[sub // 2]
                    eng(
                        AP(mp.tensor, sub * 16, [[1024, 25], [64, 16], [1, 16]]),
                        AP(t.tensor, 32 * sub * 256, [[256, 25], [16, 16], [1, 16]]),
                    )
                yM[(ro, hh)] = mp

            def unit_band(ro, hh, eng):
                """band build: 16 per-wi matmuls + 2 half copies."""
                ps = psM.tile([128, 1024], F32, tag="m")
                ym = yM[(ro, hh)]
                for wi in range(16):
                    cbase = (ro * 16 + wi) * KDIM
                    nc.tensor.matmul(
                        ps[0:KDIM, wi * 64:wi * 64 + 64],
                        ppk[0:25, cbase:cbase + KDIM],
                        ym[0:25, wi * 64:wi * 64 + 64],
                        start=True, stop=True,
                    )
                # ybig col = ro*2048 + wi*128 + sub*32 + hh*16 + (h'*4+b4)
                for half in range(2):
                    dst = AP(
                        ybig.tensor,
                        ro * 2048 + hh * 16 + half * 8 * 128,
                        [[YF, KDIM], [128, 8], [32, 4], [1, 16]],
                    )
                    cpe = (nc.vector.tensor_copy, nc.scalar.copy)[half]
                    cpe(dst, ps[0:KDIM, half * 512:half * 512 + 512])

            def unit_mac(ro, hh, engs, last=False):
                """MAC: per ct, one [128,1024] psum -> osb -> DMA. The last
                unit splits its copies/DMAs in halves to shorten the tail."""
                for ct in range(2):
                    osb = pp.tile(
                        [128, 1024], BF16,
                        name=f"osb{ro}{hh}{ct}", tag=f"osb{ro}{hh}{ct}",
                    )
                    osbs[(ro, hh, ct)] = osb
                    ps = psM.tile([128, 1024], F32, tag="m")
                    for gq in range(4):
                        g = hh * 4 + gq
                        for b4 in range(4):
                            nc.tensor.matmul(
                                ps[:, gq * 256 + b4 * 64:gq * 256 + b4 * 64 + 64],
                                xcall[:, g * 1024 + b4 * 256 + ct * 128:
                                      g * 1024 + b4 * 256 + ct * 128 + 128],
                                AP(
                                    ybig.tensor,
                                    ro * 2048 + g * 4 + b4,
                                    [[YF, KDIM], [128, 16], [32, 4]],
                                ),
                                start=True, stop=True,
                            )
                    obase = ro * 2048 + hh * 1024
                    for half in range(2):
                        eng = (nc.vector.tensor_copy, nc.scalar.copy)[
                            (half + ct) % 2
                        ]
                        eng(
                            osb[:, half * 512:half * 512 + 512],
                            ps[:, half * 512:half * 512 + 512],
                        )
                    nc.sync.dma_start(
                        out_d[ct, :, obase:obase + 1024], osb[:]
                    )

            # emission order: PE stays saturated, each unit's softmax chain
            # overlaps the next unit's encoder.
            # Every engine executes its queue IN ORDER and an unready
            # instruction head-of-line blocks everything behind it, so each
            # engine's sub-sequence here is sorted by expected ready time.
            U = [(0, 0), (1, 0), (0, 1), (1, 1)]
            unit_enc_part(*U[0], 0)
            unit_enc_part(*U[0], 1)
            unit_exp(*U[0])
            unit_enc_part(*U[1], 0)
            unit_sum(*U[0])
            unit_enc_part(*U[1], 1)
            unit_exp(*U[1])
            unit_bcast(*U[0])
            unit_enc_part(*U[2], 0)
            unit_sum(*U[1])
            unit_enc_part(*U[2], 1)
            unit_exp(*U[2])
            unit_bcast(*U[1])
            unit_band(*U[0], None)
            unit_enc_part(*U[3], 0)
            unit_sum(*U[2])
            unit_enc_part(*U[3], 1)
            unit_exp(*U[3])
            unit_bcast(*U[2])
            unit_mac(*U[0], None)
            unit_band(*U[1], None)
            unit_sum(*U[3])
            unit_bcast(*U[3])
            unit_band(*U[2], None)
            unit_mac(*U[1], None)
            unit_band(*U[3], None)
            unit_mac(*U[2], None)
            unit_mac(*U[3], None, last=True)
            ctx_mac.__exit__(None, None, None)
    nc.compile()
    return nc


_CACHE: dict[bool, object] = {}


def _get_program(with_ebias: bool):
    if with_ebias not in _CACHE:
        _CACHE[with_ebias] = build_program(with_ebias)
    return _CACHE[with_ebias]


def _prep_inputs(x, w_comp, b_comp, w_enc, b_enc):
    """Build the per-core numpy input dicts."""
    from numpy.lib.stride_tricks import sliding_window_view

    x = np.asarray(x, dtype=np.float32)
    w_comp = np.asarray(w_comp, dtype=np.float32)
    b_comp = np.asarray(b_comp, dtype=np.float32)
    w_enc = np.asarray(w_enc, dtype=np.float32)
    b_enc = np.asarray(b_enc, dtype=np.float32)

    # compress weights, channel-tiled: wp128[c', ct*32 + m] = w_comp[m, ct*128+c']
    wp128 = np.zeros((128, 64), dtype=np.float32)
    wp128[:, 0:32] = w_comp.T[0:128]
    wp128[:, 32:64] = w_comp.T[128:256]

    # encoder output channel layout: o'' = sub*32 + tap (zeros elsewhere)
    o_src = np.arange(NK)
    o2 = (o_src % 4) * 32 + o_src // 4
    sel = np.zeros((128, 4), dtype=np.float32)
    sel[o2, o_src % 4] = 1.0

    # encoder stationaries for the 4-high stacked y1:
    # wenc[32b+m, di*128 + o''] = w_enc[o, m, di, b]; cols 640: hold the
    # K=32 dj=4 slice
    wenc = np.zeros((128, 1280), dtype=np.float32)
    for di in range(5):
        for b in range(4):
            blk = np.zeros((C_MID, 128), dtype=np.float32)
            blk[:, o2] = w_enc[:, :, di, b].T
            wenc[32 * b:32 * b + 32, di * 128:di * 128 + 128] = blk
        blk = np.zeros((C_MID, 128), dtype=np.float32)
        blk[:, o2] = w_enc[:, :, di, 4].T
        wenc[0:32, 640 + di * 128:640 + di * 128 + 128] = blk
    wenc_bf = wenc.astype(BF16NP)

    # band placement matrices P_{ro,wi} [25, 120] + selt broadcast matrix
    ppackt = np.zeros((25, 3968), dtype=np.float32)
    dii = np.repeat(np.arange(5), 5)
    djj = np.tile(np.arange(5), 5)
    for ro in range(2):
        for wi in range(16):
            cols = (ro * 16 + wi) * KDIM + (ro + dii) * 20 + wi + djj
            ppackt[np.arange(25), cols] = 1.0
    ppackt[0:4, 3840:3968] = sel.T
    ppackt = ppackt.astype(BF16NP)

    with_ebias = bool(b_comp.any() or b_enc.any())

    in_maps = []
    for core in range(NCORES):
        b = core // 4
        h0 = (core % 4) * HSLICE
        xs = np.zeros((C, ROWS, WP), dtype=np.float32)
        r_lo = max(0, h0 - 2)
        r_hi = min(H, h0 + HSLICE + 2)
        xs[:, (r_lo - (h0 - 2)):(r_hi - (h0 - 2)), 2:2 + W] = x[b, :, r_lo:r_hi, :]

        # window-major MAC stationaries:
        # xcall[(r,wc), (g,b4,ct,c')] = xs[ct*128+c', 2g+r, 16b4+wc]
        A = xs.reshape(2, 128, ROWS, WP)
        W4 = sliding_window_view(A, 20, axis=3)          # [2,128,20,49,20]
        Bv = W4[:, :, :, [0, 16, 32, 48], :]             # [2,128,20,4b4,20wc]
        rows = 2 * np.arange(8)[None, :] + np.arange(6)[:, None]  # [6r, 8g]
        Cv = Bv[:, :, rows, :, :]                        # [2,128,6r,8g,4b4,20wc]
        xcall = np.ascontiguousarray(
            Cv.transpose(2, 5, 3, 4, 0, 1)
        ).reshape(KDIM, 8192).astype(BF16NP)

        xinp = np.zeros((2, 128, PADPOS + 1), dtype=np.float32)
        xinp[:, :, :PADPOS] = xs.reshape(2, 128, PADPOS)
        xb = xinp.astype(BF16NP)
        # x0/x1 column-interleaved so one DMA carries both channel-tiles'
        # prefix; the kernel reads them with stride-2 moving APs
        headp = np.zeros((128, HEADC), dtype=BF16NP)
        headp[:, 0:64] = wp128.astype(BF16NP)
        headp[:, 64:68] = sel.astype(BF16NP)
        headp[:, 68:68 + 2 * XB0] = (
            np.stack([xb[0, :, 0:XB0], xb[1, :, 0:XB0]], axis=2)
            .reshape(128, 2 * XB0)
        )
        tailp = (
            np.stack([xb[0, :, TOFF:TOFF + XT], xb[1, :, TOFF:TOFF + XT]], axis=2)
            .reshape(128, TAILC)
        )
        m = {
            "headp": headp,
            "tailp": tailp,
            "wenc": wenc_bf,
            "ppackt": ppackt,
            "xcall": xcall,
        }
        if with_ebias:
            # field[o, h, w] = b_enc[o] + conv of b_comp over the valid mask
            we = w_enc.reshape(NK, C_MID, 25)
            wb = np.einsum("omt,m->ot", we, b_comp).reshape(NK, 5, 5)
            field = np.zeros((NK, HSLICE, W), dtype=np.float32)
            for di in range(-2, 3):
                for dj in range(-2, 3):
                    hh = np.arange(h0, h0 + HSLICE)[:, None] + di
                    ww = np.arange(W)[None, :] + dj
                    valid = ((hh >= 0) & (hh < H) & (ww >= 0) & (ww < W))
                    field += (
                        wb[:, di + 2, dj + 2][:, None, None]
                        * valid[None].astype(np.float32)
                    )
            field += b_enc[:, None, None]
            # columns in (wi, h, b4) order; rows o'' = sub*32 + tap
            f = field.reshape(NK, 8, 2, 4, 16)        # (o, g, ro, b4, wi)
            f = np.transpose(f, (2, 0, 4, 1, 3))      # (ro, o, wi, g, b4)
            f = np.ascontiguousarray(f.reshape(2, NK, 512))
            fe = np.zeros((2, 128, 512), dtype=np.float32)
            fe[:, o2, :] = f
            m["ebias"] = fe
        in_maps.append(m)
    return in_maps, with_ebias


TRACE = False
LAST_RESULT = None


def kernel(x, w_comp, b_comp, w_enc, b_enc):
    global LAST_RESULT
    from concourse.bass_utils import run_bass_kernel_spmd

    in_maps, with_ebias = _prep_inputs(x, w_comp, b_comp, w_enc, b_enc)
    nc = _get_program(with_ebias)
    res = run_bass_kernel_spmd(
        nc, in_maps, core_ids=list(range(NCORES)), trace=TRACE
    )
    LAST_RESULT = res
    out = np.empty((B, C, 2 * H, 2 * W), dtype=np.float32)
    for core in range(NCORES):
        b = core // 4
        h0 = (core % 4) * HSLICE
        o = res.results[core]["out"].astype(np.float32)
        # cols: ro*2048 + g*256 + b4*64 + wi*4 + sub; sub = r1*2 + r2
        o = o.reshape(2, 128, 2, 8, 4, 16, 2, 2)   # ct c ro g b4 wi r1 r2
        o = np.transpose(o, (0, 1, 3, 2, 6, 4, 5, 7)).reshape(2, 128, 32, 128)
        out[b, :128, 2 * h0:2 * h0 + 32, :] = o[0]
        out[b, 128:, 2 * h0:2 * h0 + 32, :] = o[1]
    return out


# revision 82
# speedup vs baseline: 1.0603x; 1.0603x over previous
"""CARAFE content-aware upsampling on 8 Trainium2 NeuronCores (Bass/Tile).

Problem: x[2,256,64,64], 1x1 compress conv (256->32), 5x5 encoder conv
(32->100), pixel-shuffle(r=2) + softmax over 25 taps, then dynamic-filter
reassembly: out[b,c,2h+r1,2w+r2] = sum_k x[b,c,h+di,w+dj] * softmax_w.

Sharding: pure data-parallel over (batch, 16-row H slices) -> 8 cores.

Per-core mapping (all DMA rides one SP HWDGE queue in need-order; the
cost model serializes descriptor-gen ~630ns each on the shared HWDGE
device and charges 900ns completion-semaphore latency per DMA, so loads
are packed/coalesced and chunked only where arrival time gates compute):
  - Host ships x twice, both bf16: channel-major (packed with the tiny
    weights into a "head" + "tail" DMA pair so compress starts ~3.3us)
    and window-major xcall [120, 8192] holding the overlapping 6x20 MAC
    stationaries, so no on-device transpose/gather.
  - compress conv writes a 4-high column-shifted y1 stack (bf16), so the
    5x5 encoder conv contracts 4 dj taps per K=128 matmul (plus a K=32
    dj=4 pass): 10 matmuls per unit.
  - The back half is pipelined as 4 units U=(ro, hh): encoder conv ->
    softmax (channel-major, select-matrix matmuls for tap sums and
    reciprocal broadcast) -> band build -> MAC -> store. A unit's MAC
    only needs its own h-half band columns, so outputs stream from ~11us.
  - Band build reads the normalized weights yM [o''=sub*32+tap, pix]
    directly with partition-offset moving APs (4 sub-slices per wi), so
    no tap-major relayout copies; the block-sparse band ybig [120, 4096]
    (bf16) is written by tiny PE matmuls against host-prepared 0/1
    placement matrices (matmul writes the band's zeros too).
  - The 25-tap dynamic-filter sum runs on PE as bf16 [120]x[128] matmuls
    (stationary = xcall windows, moving = strided ybig views); psum
    halves are copied bf16 to osb on alternating Act/DVE and stored per
    (ro, ct, hh) as 8 [128,1024] DMAs; host upcasts/unscrambles.
  - A chain of dummy matmuls at t=0 ramps the PE p-state before real
    work arrives.
"""

import os
import sys

# Lift the byte-range dependency-analysis work cap so strided/partition-
# crossing APs get precise (not whole-tensor) dependency edges at build time.
os.environ.setdefault("TILE_EXHAUSTIVE_MEMORY_SHARE_CHECK", "1")

sys.path.insert(0, "/opt/trn_rl_repo")

import numpy as np
import ml_dtypes

import concourse.bacc as bacc
import concourse.bass as bass
import concourse.tile as tile
from concourse import mybir
from concourse.ap import AP

F32 = mybir.dt.float32
F32R = mybir.dt.float32r
BF16 = mybir.dt.bfloat16
BF16NP = ml_dtypes.bfloat16

# geometry
B, C, H, W = 2, 256, 64, 64
RATIO, K_UP, C_MID, ENC_K = 2, 5, 32, 5
NK = RATIO * RATIO * K_UP * K_UP  # 100
HSLICE = 16                       # output source rows per core
ROWS = HSLICE + 4                 # with 2-row halo each side
WP = W + 4                        # padded width
PADPOS = ROWS * WP                # 1360
NCORES = 8
KDIM = 120                        # 6x20 window pixels per row-pair block
YF = 4096                         # band matrix columns
NPRIME = 24                       # PE p-state priming matmuls

XB0 = 816        # head DMA covers x cols [0, 816) = rows 0-11
TOFF = 780       # tail DMA covers x cols [780, 1361] (overlap + pad col)
XT = 581         # tail cols per channel-tile
HEADC = 68 + 2 * XB0       # wp128 | selb | x0 head | x1 head
TAILC = 2 * XT
# compress chunks: chunk 0 fits the first head DMA (x cols <= 415),
# chunk 1 the second (<= 815); psum-bank limit keeps each <= 512 wide
CHUNKS = [(0, 414), (414, 400), (814, 273), (1087, PADPOS - 1087)]


def build_program(with_ebias: bool):
    nc = bacc.Bacc()
    head_d = nc.declare_dram_parameter("headp", [128, HEADC], BF16, isOutput=False)
    tail_d = nc.declare_dram_parameter("tailp", [128, TAILC], BF16, isOutput=False)
    wet_d = nc.declare_dram_parameter("wenc", [128, 1280], BF16, isOutput=False)
    pp_d = nc.declare_dram_parameter("ppackt", [25, 3968], BF16, isOutput=False)
    xc_d = nc.declare_dram_parameter("xcall", [KDIM, 8192], BF16, isOutput=False)
    if with_ebias:
        ebias_d = nc.declare_dram_parameter("ebias", [2, 128, 512], F32, isOutput=False)
    out_d = nc.declare_dram_parameter("out", [2, 128, YF], BF16, isOutput=True)

    with tile.TileContext(nc) as tc:
        # Partition-crossing APs confuse the byte-range race detector; deps
        # are tracked at tensor granularity regardless.
        tc.race_detector_enabled = False
        with (
            tc.tile_pool(name="persist", bufs=1) as pp,
            tc.tile_pool(name="psY", bufs=2, space="PSUM") as psY,   # prime/enc/sum/bcast
        ):
            psM = None  # opened after the compress pool closes (bank budget)
            # ---- PE p-state priming ----
            dummy = pp.tile([128, 128], BF16, tag="dummy")
            nc.gpsimd.memset(dummy[:], 0.0)
            for _ in range(NPRIME):
                ps = psY.tile([128, 256], F32, tag="y")
                nc.tensor.matmul(
                    ps[:, 0:128], dummy[:], dummy[:], start=True, stop=True
                )

            # ---- input loads (SP HWDGE queue, need-order) ----
            head = pp.tile([128, HEADC], BF16, tag="head")
            nc.sync.dma_start(head[:, 0:900], head_d[:, 0:900])
            nc.sync.dma_start(head[:, 900:], head_d[:, 900:])
            tail = pp.tile([128, TAILC], BF16, tag="tail")
            nc.sync.dma_start(tail[:], tail_d[:])
            wenc = pp.tile([128, 1280], BF16, tag="wenc")
            nc.sync.dma_start(wenc[:], wet_d[:])
            ppk = pp.tile([25, 3968], BF16, tag="ppackt")
            nc.sync.dma_start(ppk[:], pp_d[:])
            if with_ebias:
                ebias = []
                for ro in range(2):
                    t = pp.tile([128, 512], F32, name=f"ebias{ro}", tag=f"ebias{ro}")
                    nc.sync.dma_start(t[:], ebias_d[ro])
                    ebias.append(t)
            xcall = pp.tile([KDIM, 8192], BF16, tag="xcall")
            nc.sync.dma_start(xcall[:, 0:4096], xc_d[:, 0:4096])
            nc.sync.dma_start(xcall[:, 4096:], xc_d[:, 4096:])

            def xsrc(ct, col, n):
                """stride-2 AP into the x0/x1 column-interleaved head/tail."""
                if col + n <= XB0:
                    return AP(head.tensor, 68 + 2 * col + ct,
                              [[HEADC, 128], [2, n]])
                assert col >= TOFF
                return AP(tail.tensor, 2 * (col - TOFF) + ct,
                          [[TAILC, 128], [2, n]])

            # ---- compress conv -> 4-high column-shifted stack ----
            stk = pp.tile([128, PADPOS], BF16, tag="stk")
            ctx_cmp = tc.tile_pool(name="psC", bufs=3, space="PSUM")
            psC = ctx_cmp.__enter__()
            for ci, (off, n) in enumerate(CHUNKS):
                # both column-shifts land in one psum tile (partition blocks
                # 0/32), so a single [64, n] copy fills stack blocks 0-1
                ps = psC.tile([128, 512], F32, tag="c")
                for b in range(2):
                    for ct in range(2):
                        nc.tensor.matmul(
                            ps[32 * b:32 * b + C_MID, :n],
                            head[:, 32 * ct:32 * ct + 32],
                            xsrc(ct, off + b, n),
                            start=(ct == 0), stop=(ct == 1),
                        )
                eng = (nc.vector.tensor_copy, nc.scalar.copy)[ci % 2]
                eng(stk[0:64, off:off + n], ps[0:64, :n])
                # blocks 2/3: +2-shifted copies of blocks 0/1 (DVE 4x mode)
                s0 = max(0, off - 2)
                s1 = off + n - 2
                nc.vector.tensor_copy(stk[64:128, s0:s1], stk[0:64, s0 + 2:s1 + 2])
                if ci == len(CHUNKS) - 1:
                    nc.vector.tensor_copy(
                        stk[64:128, s1:s1 + 2], stk[0:64, s1:s1 + 2]
                    )
            ctx_cmp.__exit__(None, None, None)
            ctx_mac = tc.tile_pool(name="psM", bufs=3, space="PSUM")
            psM = ctx_mac.__enter__()

            # ---- pipelined back half: 4 units U = (ro, hh) ----
            # unit state
            y2e = {}
            rs = {}
            yM = {}
            ebias_sl = {}
            ybig = pp.tile([KDIM, YF], BF16, tag="ybig")
            osbs = {}

            pses = {}

            def unit_enc_part(ro, hh, part_sel):
                """encoder conv psum matmuls (half of them per call)."""
                u = f"{ro}{hh}"
                if part_sel == 0:
                    pses[(ro, hh)] = psY.tile(
                        [128, 256], F32, name=f"pse{ro}{hh}", tag="y"
                    )
                pse = pses[(ro, hh)]
                for i in range(5):
                    part = 1 - part_sel
                    di = i
                    nmm = part_sel * 5 + i
                    if part == 0:
                        lhsT = wenc[:, di * 128:di * 128 + 128]
                        kp = 128
                    else:
                        lhsT = wenc[0:32, 640 + di * 128:640 + di * 128 + 128]
                        kp = 32
                    rhs = AP(
                        stk.tensor,
                        (ro + di) * WP + 4 * part + hh * 8 * WP,
                        [[PADPOS, kp], [1, 16], [2 * WP, 4], [16, 4]],
                    )
                    nc.tensor.matmul(
                        pse[0:128, 0:256], lhsT, rhs,
                        start=(nmm == 0), stop=(nmm == 9),
                    )

            def unit_exp(ro, hh):
                """exp of the completed encoder psum (Act)."""
                u = f"{ro}{hh}"
                pse = pses[(ro, hh)]
                t = pp.tile([128, 256], BF16, name="y2e" + u, tag="y2e" + u)
                if with_ebias:
                    y2f = pp.tile([128, 256], F32, name="y2f" + u, tag="y2f" + u)
                    nc.vector.scalar_tensor_tensor(
                        y2f[:], pse[0:128, 0:256], 1.0,
                        AP(ebias[ro].tensor, hh * 16, [[512, 128], [32, 16], [1, 16]]),
                        op0=mybir.AluOpType.mult, op1=mybir.AluOpType.add,
                    )
                    nc.scalar.activation(
                        t[:], y2f[:], mybir.ActivationFunctionType.Exp
                    )
                else:
                    nc.scalar.activation(
                        t[:], pse[0:128, 0:256], mybir.ActivationFunctionType.Exp
                    )
                y2e[(ro, hh)] = t

            def unit_sum(ro, hh):
                """tap-sum select matmul + reciprocal (DVE)."""
                u = f"{ro}{hh}"
                pss = psY.tile([128, 256], F32, tag="y")
                nc.tensor.matmul(
                    pss[0:4, 0:256], head[:, 64:68], y2e[(ro, hh)][:],
                    start=True, stop=True,
                )
                r = pp.tile([4, 256], BF16, name="rs" + u, tag="rs" + u)
                with nc.allow_low_precision(reason="softmax sum recip in bf16"):
                    nc.vector.reciprocal(r[:], pss[0:4, 0:256])
                rs[(ro, hh)] = r

            def unit_bcast(ro, hh):
                """recip broadcast matmul + normalize TT (DVE)."""
                u = f"{ro}{hh}"
                psb = psY.tile([128, 256], F32, tag="y")
                nc.tensor.matmul(
                    psb[0:128, 0:256], ppk[0:4, 3840:3968], rs[(ro, hh)][:],
                    start=True, stop=True,
                )
                t = pp.tile([128, 256], BF16, name="yM" + u, tag="yM" + u)
                nc.vector.tensor_tensor(
                    t[:], y2e[(ro, hh)][:], psb[0:128, 0:256],
                    op=mybir.AluOpType.mult,
                )
                # tap-major relayout (matmul operands must share a base
                # partition, so the sub-slices move to partitions 0-24):
                # ymp[tap, wi*64 + sub*16 + j] = yM[32sub+tap, wi*16+j]
                mp = pp.tile([25, 1024], BF16, name="ymp" + u, tag="ymp" + u)
                for sub in range(4):
                    # 2 on DVE (4x) + 2 on the otherwise-idle Pool: Act/DVE
                    # copy throughput paces the tail, Pool does not
                    eng = (nc.vector.tensor_copy, nc.gpsimd.tensor_copy)[sub // 2]
                    eng(
                        AP(mp.tensor, sub * 16, [[1024, 25], [64, 16], [1, 16]]),
                        AP(t.tensor, 32 * sub * 256, [[256, 25], [16, 16], [1, 16]]),
                    )
                yM[(ro, hh)] = mp

            def unit_band(ro, hh, eng):
                """band build: 16 per-wi matmuls + 2 half copies."""
                ps = psM.tile([128, 1024], F32, tag="m")
                ym = yM[(ro, hh)]
                for wi in range(16):
                    cbase = (ro * 16 + wi) * KDIM
                    nc.tensor.matmul(
                        ps[0:KDIM, wi * 64:wi * 64 + 64],
                        ppk[0:25, cbase:cbase + KDIM],
                        ym[0:25, wi * 64:wi * 64 + 64],
                        start=True, stop=True,
                    )
                # ybig col = ro*2048 + wi*128 + sub*32 + hh*16 + (h'*4+b4)
                for half in range(2):
                    dst = AP(
                        ybig.tensor,
                        ro * 2048 + hh * 16 + half * 8 * 128,
                        [[YF, KDIM], [128, 8], [32, 4], [1, 16]],
                    )
                    cpe = (nc.vector.tensor_copy, nc.scalar.copy)[half]
                    cpe(dst, ps[0:KDIM, half * 512:half * 512 + 512])

            def unit_mac(ro, hh, engs, last=False):
                """MAC: per ct, one [128,1024] psum -> osb -> DMA. The last
                unit splits its copies/DMAs in halves to shorten the tail."""
                for ct in range(2):
                    osb = pp.tile(
                        [128, 1024], BF16,
                        name=f"osb{ro}{hh}{ct}", tag=f"osb{ro}{hh}{ct}",
                    )
                    osbs[(ro, hh, ct)] = osb
                    ps = psM.tile([128, 1024], F32, tag="m")
                    for gq in range(4):
                        g = hh * 4 + gq
                        for b4 in range(4):
                            nc.tensor.matmul(
                                ps[:, gq * 256 + b4 * 64:gq * 256 + b4 * 64 + 64],
                                xcall[:, g * 1024 + b4 * 256 + ct * 128:
                                      g * 1024 + b4 * 256 + ct * 128 + 128],
                                AP(
                                    ybig.tensor,
                                    ro * 2048 + g * 4 + b4,
                                    [[YF, KDIM], [128, 16], [32, 4]],
                                ),
                                start=True, stop=True,
                            )
                    obase = ro * 2048 + hh * 1024
                    for half in range(2):
                        eng = (nc.vector.tensor_copy, nc.scalar.copy)[
                            (half + ct) % 2
                        ]
                        eng(
                            osb[:, half * 512:half * 512 + 512],
                            ps[:, half * 512:half * 512 + 512],
                        )
                    nc.sync.dma_start(
                        out_d[ct, :, obase:obase + 1024], osb[:]
                    )

            # emission order: PE stays saturated, each unit's softmax chain
            # overlaps the next unit's encoder.
            # Every engine executes its queue IN ORDER and an unready
            # instruction head-of-line blocks everything behind it, so each
            # engine's sub-sequence here is sorted by expected ready time.
            U = [(0, 0), (1, 0), (0, 1), (1, 1)]
            unit_enc_part(*U[0], 0)
            unit_enc_part(*U[0], 1)
            unit_exp(*U[0])
            unit_enc_part(*U[1], 0)
            unit_sum(*U[0])
            unit_enc_part(*U[1], 1)
            unit_exp(*U[1])
            unit_bcast(*U[0])
            unit_enc_part(*U[2], 0)
            unit_sum(*U[1])
            unit_enc_part(*U[2], 1)
            unit_exp(*U[2])
            unit_bcast(*U[1])
            unit_band(*U[0], None)
            unit_enc_part(*U[3], 0)
            unit_sum(*U[2])
            unit_enc_part(*U[3], 1)
            unit_exp(*U[3])
            unit_bcast(*U[2])
            unit_mac(*U[0], None)
            unit_band(*U[1], None)
            unit_sum(*U[3])
            unit_bcast(*U[3])
            unit_band(*U[2], None)
            unit_mac(*U[1], None)
            unit_band(*U[3], None)
            unit_mac(*U[2], None)
            unit_mac(*U[3], None, last=True)
            ctx_mac.__exit__(None, None, None)
    nc.compile()
    return nc


_CACHE: dict[bool, object] = {}


def _get_program(with_ebias: bool):
    if with_ebias not in _CACHE:
        _CACHE[with_ebias] = build_program(with_ebias)
    return _CACHE[with_ebias]


def _prep_inputs(x, w_comp, b_comp, w_enc, b_enc):
    """Build the per-core numpy input dicts."""
    from numpy.lib.stride_tricks import sliding_window_view

    x = np.asarray(x, dtype=np.float32)
    w_comp = np.asarray(w_comp, dtype=np.float32)
    b_comp = np.asarray(b_comp, dtype=np.float32)
    w_enc = np.asarray(w_enc, dtype=np.float32)
    b_enc = np.asarray(b_enc, dtype=np.float32)

    # compress weights, channel-tiled: wp128[c', ct*32 + m] = w_comp[m, ct*128+c']
    wp128 = np.zeros((128, 64), dtype=np.float32)
    wp128[:, 0:32] = w_comp.T[0:128]
    wp128[:, 32:64] = w_comp.T[128:256]

    # encoder output channel layout: o'' = sub*32 + tap (zeros elsewhere)
    o_src = np.arange(NK)
    o2 = (o_src % 4) * 32 + o_src // 4
    sel = np.zeros((128, 4), dtype=np.float32)
    sel[o2, o_src % 4] = 1.0

    # encoder stationaries for the 4-high stacked y1:
    # wenc[32b+m, di*128 + o''] = w_enc[o, m, di, b]; cols 640: hold the
    # K=32 dj=4 slice
    wenc = np.zeros((128, 1280), dtype=np.float32)
    for di in range(5):
        for b in range(4):
            blk = np.zeros((C_MID, 128), dtype=np.float32)
            blk[:, o2] = w_enc[:, :, di, b].T
            wenc[32 * b:32 * b + 32, di * 128:di * 128 + 128] = blk
        blk = np.zeros((C_MID, 128), dtype=np.float32)
        blk[:, o2] = w_enc[:, :, di, 4].T
        wenc[0:32, 640 + di * 128:640 + di * 128 + 128] = blk
    wenc_bf = wenc.astype(BF16NP)

    # band placement matrices P_{ro,wi} [25, 120] + selt broadcast matrix
    ppackt = np.zeros((25, 3968), dtype=np.float32)
    dii = np.repeat(np.arange(5), 5)
    djj = np.tile(np.arange(5), 5)
    for ro in range(2):
        for wi in range(16):
            cols = (ro * 16 + wi) * KDIM + (ro + dii) * 20 + wi + djj
            ppackt[np.arange(25), cols] = 1.0
    ppackt[0:4, 3840:3968] = sel.T
    ppackt = ppackt.astype(BF16NP)

    with_ebias = bool(b_comp.any() or b_enc.any())

    in_maps = []
    for core in range(NCORES):
        b = core // 4
        h0 = (core % 4) * HSLICE
        xs = np.zeros((C, ROWS, WP), dtype=np.float32)
        r_lo = max(0, h0 - 2)
        r_hi = min(H, h0 + HSLICE + 2)
        xs[:, (r_lo - (h0 - 2)):(r_hi - (h0 - 2)), 2:2 + W] = x[b, :, r_lo:r_hi, :]

        # window-major MAC stationaries:
        # xcall[(r,wc), (g,b4,ct,c')] = xs[ct*128+c', 2g+r, 16b4+wc]
        A = xs.reshape(2, 128, ROWS, WP)
        W4 = sliding_window_view(A, 20, axis=3)          # [2,128,20,49,20]
        Bv = W4[:, :, :, [0, 16, 32, 48], :]             # [2,128,20,4b4,20wc]
        rows = 2 * np.arange(8)[None, :] + np.arange(6)[:, None]  # [6r, 8g]
        Cv = Bv[:, :, rows, :, :]                        # [2,128,6r,8g,4b4,20wc]
        xcall = np.ascontiguousarray(
            Cv.transpose(2, 5, 3, 4, 0, 1)
        ).reshape(KDIM, 8192).astype(BF16NP)

        xinp = np.zeros((2, 128, PADPOS + 1), dtype=np.float32)
        xinp[:, :, :PADPOS] = xs.reshape(2, 128, PADPOS)
        xb = xinp.astype(BF16NP)
        # x0/x1 column-interleaved so one DMA carries both channel-tiles'
        # prefix; the kernel reads them with stride-2 moving APs
        headp = np.zeros((128, HEADC), dtype=BF16NP)
        headp[:, 0:64] = wp128.astype(BF16NP)
        headp[:, 64:68] = sel.astype(BF16NP)
        headp[:, 68:68 + 2 * XB0] = (
            np.stack([xb[0, :, 0:XB0], xb[1, :, 0:XB0]], axis=2)
            .reshape(128, 2 * XB0)
        )
        tailp = (
            np.stack([xb[0, :, TOFF:TOFF + XT], xb[1, :, TOFF:TOFF + XT]], axis=2)
            .reshape(128, TAILC)
        )
        m = {
            "headp": headp,
            "tailp": tailp,
            "wenc": wenc_bf,
            "ppackt": ppackt,
            "xcall": xcall,
        }
        if with_ebias:
            # field[o, h, w] = b_enc[o] + conv of b_comp over the valid mask
            we = w_enc.reshape(NK, C_MID, 25)
            wb = np.einsum("omt,m->ot", we, b_comp).reshape(NK, 5, 5)
            field = np.zeros((NK, HSLICE, W), dtype=np.float32)
            for di in range(-2, 3):
                for dj in range(-2, 3):
                    hh = np.arange(h0, h0 + HSLICE)[:, None] + di
                    ww = np.arange(W)[None, :] + dj
                    valid = ((hh >= 0) & (hh < H) & (ww >= 0) & (ww < W))
                    field += (
                        wb[:, di + 2, dj + 2][:, None, None]
                        * valid[None].astype(np.float32)
                    )
            field += b_enc[:, None, None]
            # columns in (wi, h, b4) order; rows o'' = sub*32 + tap
            f = field.reshape(NK, 8, 2, 4, 16)        # (o, g, ro, b4, wi)
            f = np.transpose(f, (2, 0, 4, 1, 3))      # (ro, o, wi, g, b4)
            f = np.ascontiguousarray(f.reshape(2, NK, 512))
            fe = np.zeros((2, 128, 512), dtype=np.float32)
            fe[:, o2, :] = f
            m["ebias"] = fe
        in_maps.append(m)
    return in_maps, with_ebias


TRACE = False
LAST_RESULT = None


def kernel(x, w_comp, b_comp, w_enc, b_enc):
    global LAST_RESULT
    from concourse.bass_utils import run_bass_kernel_spmd

    in_maps, with_ebias = _prep_inputs(x, w_comp, b_comp, w_enc, b_enc)
    nc = _get_program(with_ebias)
    res = run_bass_kernel_spmd(
        nc, in_maps, core_ids=list(range(NCORES)), trace=TRACE
    )
    LAST_RESULT = res
    out = np.empty((B, C, 2 * H, 2 * W), dtype=np.float32)
    for core in range(NCORES):
        b = core // 4
        h0 = (core % 4) * HSLICE
        o = res.results[core]["out"].astype(np.float32)
        # cols: ro*2048 + g*256 + b4*64 + wi*4 + sub; sub = r1*2 + r2
        o = o.reshape(2, 128, 2, 8, 4, 16, 2, 2)   # ct c ro g b4 wi r1 r2
        o = np.transpose(o, (0, 1, 3, 2, 6, 4, 5, 7)).reshape(2, 128, 32, 128)
        out[b, :128, 2 * h0:2 * h0 + 32, :] = o[0]
        out[b, 128:, 2 * h0:2 * h0 + 32, :] = o[1]
    return out
